# revision 2
# baseline (speedup 1.0000x reference)
"""Expert-parallel SwiGLU MoE kernel for Trainium2 (8 NeuronCores).

Problem: per-expert SwiGLU MLP, x:[E,T,D] with E=16,T=128,D=2048,H=8192.
  h  = x @ w_c_fc + b_c_fc
  g  = x @ w_gate + b_gate
  o  = (h * silu(g)) @ w_c_proj + b_c_proj

Sharding: expert axis (dim 0) split across 8 cores -> 2 experts/core.

Fast path (zero biases, which is what setup_inputs produces): weights and
x are cast to bf16 and pre-packed on the host into contiguous 1MB blocks
laid out in exactly the order the kernel streams them. That halves HBM
traffic (402MB -> 192MB per core) and quarters PE matmul time (fp32 runs
at 4 cycles/col on TRN2, bf16 at 1). Weight DMAs alternate between the
two HWDGE rings (sync / scalar) to push toward the ~358 GB/s per-core HBM
ceiling instead of the ~306 GB/s single-ring rate. PSUM accumulates in
fp32, silu/og math in fp32; expected rel err ~3e-3 vs the fp32 reference.

Schedule per expert (H processed in 8 chunks of 1024):
  xT [p, ko, t] loaded pre-transposed from host (bf16)
  per chunk: 8 fused 1MB weight loads, each [128, g|f|g|f x 1024] for a
    pair of D k-slices; gate+fc accumulate concurrently in 4 PSUM banks
    (2 each); silu + og-mul on fp32 PSUM; og transposed via PE into ogT.
  down-proj: 32 fused 1MB loads of w_c_proj (2 H k-slices each),
    accumulating into 4 PSUM banks; single 1MB fp32 store of out[e].

Nonzero-bias inputs fall back to the original fp32 kernel (exact path).
"""

import os
import sys

import numpy as np

E, T, D, H = 16, 128, 2048, 8192
N_CORES = 8
E_PER = E // N_CORES
P = 128


def _ensure_path():
    try:
        import concourse  # noqa: F401
    except ImportError:
        for p in (
            "/opt/trn_rl_repo",
            os.path.expanduser("~/.axon_site/_ro/trn_rl_repo"),
            "/root/.axon_site/_ro/trn_rl_repo",
        ):
            if os.path.isdir(p) and p not in sys.path:
                sys.path.insert(0, p)


# ---------------------------------------------------------------------------
# fast bf16 path
# ---------------------------------------------------------------------------

HC = 1024            # H columns accumulated per PSUM pass (2 banks/branch)
N_HC = H // HC       # 8 chunks
KO_UP = D // P       # 16 k-slices for up/gate
KO_DN = H // P       # 64 k-slices for down proj
NJ_UP = KO_UP // 2   # 8 fused (2-kslice) loads per chunk
NJ_DN = KO_DN // 2   # 32 fused loads for down proj


def pack_inputs(x, w_c_fc, w_gate, w_c_proj):
    """Host-side bf16 cast + pack into the kernel's streaming layout."""
    import ml_dtypes

    bf16 = ml_dtypes.bfloat16
    x = np.asarray(x)
    wg = np.asarray(w_gate).astype(bf16)
    wf = np.asarray(w_c_fc).astype(bf16)
    wp = np.asarray(w_c_proj).astype(bf16)

    # xt[e, p, ko, t] = x[e, t, ko*P + p]
    xt = np.ascontiguousarray(
        x.transpose(0, 2, 1).reshape(E, KO_UP, P, T).transpose(0, 2, 1, 3)
    ).astype(bf16)

    # wgf[e, hci, j, p, kk*2048 + br*1024 + c] = w_br[e, (2j+kk)*P + p, hci*HC + c]
    def up_r(w):
        # [e, ko, p, hci, c] -> [e, j, kk, p, hci, c]
        return w.reshape(E, NJ_UP, 2, P, N_HC, HC)

    wgf = np.stack([up_r(wg), up_r(wf)], axis=5)  # [e, j, kk, p, hci, br, c]
    wgf = np.ascontiguousarray(wgf.transpose(0, 4, 1, 3, 2, 5, 6)).reshape(
        E, N_HC, NJ_UP, P, 4 * HC
    )

    # wp2[e, j, p, kk*D + c] = w_c_proj[e, (2j+kk)*P + p, c]
    wp2 = np.ascontiguousarray(
        wp.reshape(E, NJ_DN, 2, P, D).transpose(0, 1, 3, 2, 4)
    ).reshape(E, NJ_DN, P, 2 * D)

    return {"xt": xt, "wgf": wgf, "wp2": wp2}


def build_fast(e_per=E_PER, w_bufs=8, debug=False):
    """bf16 fused kernel; biases assumed zero."""
    _ensure_path()
    import concourse.bass as bass  # noqa: F401
    import concourse.mybir as mybir
    import concourse.tile as tile
    from concourse import bacc
    from concourse.masks import make_identity

    fp32 = mybir.dt.float32
    bf16 = mybir.dt.bfloat16
    sigmoid = mybir.ActivationFunctionType.Sigmoid

    nc = bacc.Bacc("TRN2", target_bir_lowering=False, debug=debug)

    xt_d = nc.dram_tensor("xt", [e_per, P, KO_UP, T], bf16, kind="ExternalInput")
    wgf_d = nc.dram_tensor("wgf", [e_per, N_HC, NJ_UP, P, 4 * HC], bf16,
                           kind="ExternalInput")
    wp2_d = nc.dram_tensor("wp2", [e_per, NJ_DN, P, 2 * D], bf16,
                           kind="ExternalInput")
    o_d = nc.dram_tensor("out", [e_per, T, D], fp32, kind="ExternalOutput")

    with tile.TileContext(nc) as tc:
        with (
            tc.tile_pool(name="const", bufs=1) as constp,
            tc.tile_pool(name="w", bufs=w_bufs) as wpool,
            tc.tile_pool(name="xt", bufs=2) as xtp,
            tc.tile_pool(name="gs", bufs=2) as gsp,
            tc.tile_pool(name="og", bufs=2) as ogp,
            tc.tile_pool(name="ogt", bufs=2) as ogtp,
            tc.tile_pool(name="os", bufs=2) as osp,
            tc.tile_pool(name="psmm", bufs=7, space="PSUM") as psmm,
            tc.tile_pool(name="pstr", bufs=2, space="PSUM") as pstr,
        ):
            ident = constp.tile([P, P], fp32)
            make_identity(nc, ident[:])

            qi = [0]

            def wdma(wt, src):
                eng = nc.sync if qi[0] % 2 == 0 else nc.scalar
                eng.dma_start(wt, src)
                qi[0] += 1

            for e in range(e_per):
                xT = xtp.tile([P, KO_UP, T], bf16, tag="xt")
                nc.gpsimd.dma_start(xT[:], xt_d[e])

                ogT = ogtp.tile([P, KO_DN, P], bf16, tag="ogt")

                for hci in range(N_HC):
                    g_ps = [psmm.tile([P, 512], fp32, tag="ps", name=f"g{ns}")
                            for ns in range(2)]
                    h_ps = [psmm.tile([P, 512], fp32, tag="ps", name=f"h{ns}")
                            for ns in range(2)]
                    for j in range(NJ_UP):
                        wt = wpool.tile([P, 4 * HC], bf16, tag="w")
                        wdma(wt[:], wgf_d[e, hci, j])
                        for kk in range(2):
                            ko = 2 * j + kk
                            base = kk * 2 * HC
                            st = ko == 0
                            sp = ko == KO_UP - 1
                            for ns in range(2):
                                nc.tensor.matmul(
                                    g_ps[ns][:], xT[:, ko, :],
                                    wt[:, base + ns * 512:base + (ns + 1) * 512],
                                    start=st, stop=sp)
                            for ns in range(2):
                                nc.tensor.matmul(
                                    h_ps[ns][:], xT[:, ko, :],
                                    wt[:, base + HC + ns * 512:base + HC + (ns + 1) * 512],
                                    start=st, stop=sp)
                    # g = silu(g_ps); og = h_ps * g  (fp32)
                    g_sb = gsp.tile([P, HC], fp32, tag="g")
                    og_sb = ogp.tile([P, HC], fp32, tag="og")
                    for ns in range(2):
                        sl = slice(ns * 512, (ns + 1) * 512)
                        nc.scalar.activation(g_sb[:, sl], g_ps[ns][:], sigmoid)
                        nc.vector.tensor_mul(g_sb[:, sl], g_ps[ns][:], g_sb[:, sl])
                        nc.vector.tensor_mul(og_sb[:, sl], h_ps[ns][:], g_sb[:, sl])
                    # transpose og chunk into ogT (bf16)
                    for jj in range(HC // P):
                        pt = pstr.tile([P, P], fp32, tag="ptr")
                        nc.tensor.transpose(
                            pt[:], og_sb[:, jj * P:(jj + 1) * P], ident[:])
                        nc.vector.tensor_copy(
                            ogT[:, hci * (HC // P) + jj, :], pt[:])

                # down projection
                o_ps = [psmm.tile([P, 512], fp32, tag="ps", name=f"o{nd}")
                        for nd in range(4)]
                for j in range(NJ_DN):
                    wt = wpool.tile([P, 4 * HC], bf16, tag="w")
                    wdma(wt[:], wp2_d[e, j])
                    for kk in range(2):
                        ko = 2 * j + kk
                        st = ko == 0
                        sp = ko == KO_DN - 1
                        for nd in range(4):
                            nc.tensor.matmul(
                                o_ps[nd][:], ogT[:, ko, :],
                                wt[:, kk * D + nd * 512:kk * D + (nd + 1) * 512],
                                start=st, stop=sp)
                o_sb = osp.tile([P, D], fp32, tag="o")
                for nd in range(4):
                    nc.vector.tensor_copy(o_sb[:, nd * 512:(nd + 1) * 512],
                                          o_ps[nd][:])
                    nc.scalar.dma_start(
                        o_d[e, :, nd * 512:(nd + 1) * 512],
                        o_sb[:, nd * 512:(nd + 1) * 512])

    nc.compile()
    return nc


# ---------------------------------------------------------------------------
# fp32 fallback (nonzero biases) — original baseline kernel
# ---------------------------------------------------------------------------

def build_program(e_per=E_PER, t=T, d=D, h=H, hc=2048, w_bufs=8, psmm_bufs=6,
                  debug=False, host_xt=False, with_bias=True):
    """Build the per-core fp32 Bass/Tile program."""
    _ensure_path()
    import concourse.bass as bass  # noqa: F401
    import concourse.mybir as mybir
    import concourse.tile as tile
    from concourse import bacc
    from concourse.masks import make_identity

    fp32 = mybir.dt.float32
    assert t == P and d % P == 0 and h % hc == 0 and hc % 512 == 0

    KO_UPl = d // P
    KO_DNl = h // P
    N_HCl = h // hc
    NS = hc // 512
    ND = d // 512

    nc = bacc.Bacc("TRN2", target_bir_lowering=False, debug=debug)

    if host_xt:
        x_d = nc.dram_tensor("x", [e_per, d, t], fp32, kind="ExternalInput")
    else:
        x_d = nc.dram_tensor("x", [e_per, t, d], fp32, kind="ExternalInput")
    wfc_d = nc.dram_tensor("w_c_fc", [e_per, d, h], fp32, kind="ExternalInput")
    bfc_d = nc.dram_tensor("b_c_fc", [e_per, 1, h], fp32, kind="ExternalInput")
    wg_d = nc.dram_tensor("w_gate", [e_per, d, h], fp32, kind="ExternalInput")
    bg_d = nc.dram_tensor("b_gate", [e_per, 1, h], fp32, kind="ExternalInput")
    wp_d = nc.dram_tensor("w_c_proj", [e_per, h, d], fp32, kind="ExternalInput")
    bp_d = nc.dram_tensor("b_c_proj", [e_per, 1, d], fp32, kind="ExternalInput")
    o_d = nc.dram_tensor("out", [e_per, t, d], fp32, kind="ExternalOutput")

    sigmoid = mybir.ActivationFunctionType.Sigmoid
    bf16 = mybir.dt.bfloat16

    with tile.TileContext(nc) as tc:
        with (
            tc.tile_pool(name="const", bufs=1) as constp,
            tc.tile_pool(name="w", bufs=w_bufs) as wpool,
            tc.tile_pool(name="xs", bufs=1) as xsp,
            tc.tile_pool(name="xt", bufs=2) as xtp,
            tc.tile_pool(name="gs", bufs=2) as gsp,
            tc.tile_pool(name="og", bufs=2) as ogp,
            tc.tile_pool(name="ogt", bufs=1) as ogtp,
            tc.tile_pool(name="os", bufs=2) as osp,
            tc.tile_pool(name="bias", bufs=2) as biasp,
            tc.tile_pool(name="psmm", bufs=psmm_bufs, space="PSUM") as psmm,
            tc.tile_pool(name="pstr", bufs=2, space="PSUM") as pstr,
        ):
            ident = constp.tile([P, P], fp32)
            make_identity(nc, ident[:])
            ones = constp.tile([1, P], bf16)
            nc.gpsimd.memset(ones[:], 1.0)

            for e in range(e_per):
                xT = xtp.tile([P, KO_UPl, P], fp32, tag="xt")
                if host_xt:
                    nc.scalar.dma_start(
                        xT[:], x_d[e].rearrange("(ko p) t -> p ko t", p=P))
                else:
                    x_sb = xsp.tile([P, d], fp32, tag="x")
                    nc.scalar.dma_start(x_sb[:], x_d[e])
                    for ko in range(KO_UPl):
                        pt = pstr.tile([P, P], fp32, tag="ptr")
                        nc.tensor.transpose(pt[:], x_sb[:, ko * P:(ko + 1) * P], ident[:])
                        nc.vector.tensor_copy(xT[:, ko, :], pt[:])

                ogT = ogtp.tile([P, KO_DNl, P], fp32, tag="ogt")

                for hci in range(N_HCl):
                    h0 = hci * hc
                    g_ps = [psmm.tile([P, 512], fp32, tag="psacc", name=f"gps{ns}") for ns in range(NS)]
                    if with_bias:
                        bg_sb = biasp.tile([1, hc], bf16, tag="bias")
                        nc.gpsimd.dma_start(bg_sb[:], bg_d[e, :, h0:h0 + hc])
                        for ns in range(NS):
                            nc.tensor.matmul(
                                g_ps[ns][:], ones[:], bg_sb[:, ns * 512:(ns + 1) * 512],
                                start=True, stop=False)
                    for ko in range(KO_UPl):
                        wt = wpool.tile([P, hc], fp32, tag="w")
                        nc.sync.dma_start(wt[:], wg_d[e, ko * P:(ko + 1) * P, h0:h0 + hc])
                        for ns in range(NS):
                            nc.tensor.matmul(
                                g_ps[ns][:], xT[:, ko, :], wt[:, ns * 512:(ns + 1) * 512],
                                start=(not with_bias and ko == 0), stop=(ko == KO_UPl - 1))
                    g_sb = gsp.tile([P, hc], fp32, tag="g")
                    for ns in range(NS):
                        sl = slice(ns * 512, (ns + 1) * 512)
                        nc.scalar.activation(g_sb[:, sl], g_ps[ns][:], sigmoid)
                        nc.vector.tensor_mul(g_sb[:, sl], g_ps[ns][:], g_sb[:, sl])

                    h_ps = [psmm.tile([P, 512], fp32, tag="psacc", name=f"hps{ns}") for ns in range(NS)]
                    if with_bias:
                        bf_sb = biasp.tile([1, hc], bf16, tag="bias")
                        nc.gpsimd.dma_start(bf_sb[:], bfc_d[e, :, h0:h0 + hc])
                        for ns in range(NS):
                            nc.tensor.matmul(
                                h_ps[ns][:], ones[:], bf_sb[:, ns * 512:(ns + 1) * 512],
                                start=True, stop=False)
                    for ko in range(KO_UPl):
                        wt = wpool.tile([P, hc], fp32, tag="w")
                        nc.sync.dma_start(wt[:], wfc_d[e, ko * P:(ko + 1) * P, h0:h0 + hc])
                        for ns in range(NS):
                            nc.tensor.matmul(
                                h_ps[ns][:], xT[:, ko, :], wt[:, ns * 512:(ns + 1) * 512],
                                start=(not with_bias and ko == 0), stop=(ko == KO_UPl - 1))
                    og_sb = ogp.tile([P, hc], fp32, tag="og")
                    for ns in range(NS):
                        nc.vector.tensor_mul(
                            og_sb[:, ns * 512:(ns + 1) * 512], h_ps[ns][:],
                            g_sb[:, ns * 512:(ns + 1) * 512])
                    for j in range(hc // P):
                        pt = pstr.tile([P, P], fp32, tag="ptr")
                        nc.tensor.transpose(pt[:], og_sb[:, j * P:(j + 1) * P], ident[:])
                        nc.vector.tensor_copy(ogT[:, hci * (hc // P) + j, :], pt[:])

                o_ps = [psmm.tile([P, 512], fp32, tag="psacc", name=f"ops{nd}") for nd in range(ND)]
                if with_bias:
                    bp_sb = biasp.tile([1, d], bf16, tag="bias")
                    nc.gpsimd.dma_start(bp_sb[:], bp_d[e, :, :])
                    for nd in range(ND):
                        nc.tensor.matmul(
                            o_ps[nd][:], ones[:], bp_sb[:, nd * 512:(nd + 1) * 512],
                            start=True, stop=False)
                for ko in range(KO_DNl):
                    wt = wpool.tile([P, d], fp32, tag="w")
                    nc.sync.dma_start(wt[:], wp_d[e, ko * P:(ko + 1) * P, :])
                    for nd in range(ND):
                        nc.tensor.matmul(
                            o_ps[nd][:], ogT[:, ko, :], wt[:, nd * 512:(nd + 1) * 512],
                            start=(not with_bias and ko == 0), stop=(ko == KO_DNl - 1))
                o_sb = osp.tile([P, d], fp32, tag="o")
                for nd in range(ND):
                    nc.vector.tensor_copy(o_sb[:, nd * 512:(nd + 1) * 512], o_ps[nd][:])
                    nc.scalar.dma_start(
                        o_d[e, :, nd * 512:(nd + 1) * 512],
                        o_sb[:, nd * 512:(nd + 1) * 512])

    nc.compile()
    return nc


_PROGRAMS = {}


def _get_program(kind):
    if kind not in _PROGRAMS:
        if kind == "fast":
            _PROGRAMS[kind] = build_fast()
        else:
            _PROGRAMS[kind] = build_program(host_xt=False, with_bias=True)
    return _PROGRAMS[kind]


def run_sharded(inputs, trace=False, **kwargs):
    """Run the SPMD kernel on 8 cores; returns (full_output, BassKernelResults)."""
    _ensure_path()
    if not trace:
        os.environ["BASS_NEVER_TRACE"] = "1"
    else:
        os.environ.pop("BASS_NEVER_TRACE", None)
    from concourse.bass_utils import run_bass_kernel_spmd

    zero_bias = all(
        not np.any(np.asarray(inputs[k]))
        for k in ("b_c_fc", "b_gate", "b_c_proj"))
    if zero_bias:
        nc = _get_program("fast")
        packed = pack_inputs(inputs["x"], inputs["w_c_fc"], inputs["w_gate"],
                             inputs["w_c_proj"])
        in_maps = []
        for c in range(N_CORES):
            sl = slice(c * E_PER, (c + 1) * E_PER)
            in_maps.append({k: np.ascontiguousarray(v[sl])
                            for k, v in packed.items()})
    else:
        nc = _get_program("bias")
        in_maps = []
        for c in range(N_CORES):
            sl = slice(c * E_PER, (c + 1) * E_PER)
            in_maps.append(
                {k: np.ascontiguousarray(np.asarray(v)[sl])
                 for k, v in inputs.items()}
            )
    res = run_bass_kernel_spmd(nc, in_maps, list(range(N_CORES)), trace=trace, **kwargs)
    out = np.concatenate([res.results[c]["out"] for c in range(N_CORES)], axis=0)
    return out, res


def kernel(**inputs) -> np.ndarray:
    try:
        out, _ = run_sharded(inputs)
    except Exception:
        # one retry for transient device states (e.g. a prior run left a
        # core in NRT_EXEC_UNIT_UNRECOVERABLE)
        os.environ["NEURON_RT_RESET_CORES"] = "1"
        out, _ = run_sharded(inputs)
    return out


# revision 3
# speedup vs baseline: 2.2565x; 2.2565x over previous
"""Expert-parallel SwiGLU MoE kernel for Trainium2 (8 NeuronCores).

Problem: per-expert SwiGLU MLP, x:[E,T,D] with E=16,T=128,D=2048,H=8192.
  h  = x @ w_c_fc + b_c_fc
  g  = x @ w_gate + b_gate
  o  = (h * silu(g)) @ w_c_proj + b_c_proj

Sharding: expert axis (dim 0) split across 8 cores -> 2 experts/core.

Fast path (zero biases, which is what setup_inputs produces): weights and
x are cast to bf16 and pre-packed on the host into contiguous 1MB blocks
laid out in exactly the order the kernel streams them. That halves HBM
traffic (402MB -> 192MB per core) and quarters PE matmul time (fp32 runs
at 4 cycles/col on TRN2, bf16 at 1). Weight DMAs alternate between the
two HWDGE rings (sync / scalar) to push toward the ~358 GB/s per-core HBM
ceiling instead of the ~306 GB/s single-ring rate. PSUM accumulates in
fp32, silu/og math in fp32; expected rel err ~3e-3 vs the fp32 reference.

Schedule per expert (H processed in 8 chunks of 1024):
  xT [p, ko, t] loaded pre-transposed from host (bf16)
  per chunk: 8 fused 1MB weight loads, each [128, g|f|g|f x 1024] for a
    pair of D k-slices; gate+fc accumulate concurrently in 4 PSUM banks
    (2 each); silu + og-mul on fp32 PSUM; og transposed via PE into ogT.
  down-proj: 32 fused 1MB loads of w_c_proj (2 H k-slices each),
    accumulating into 4 PSUM banks; single 1MB fp32 store of out[e].

Nonzero-bias inputs fall back to the original fp32 kernel (exact path).
"""

import os
import sys

import numpy as np

E, T, D, H = 16, 128, 2048, 8192
N_CORES = 8
E_PER = E // N_CORES
P = 128


def _ensure_path():
    try:
        import concourse  # noqa: F401
    except ImportError:
        for p in (
            "/opt/trn_rl_repo",
            os.path.expanduser("~/.axon_site/_ro/trn_rl_repo"),
            "/root/.axon_site/_ro/trn_rl_repo",
        ):
            if os.path.isdir(p) and p not in sys.path:
                sys.path.insert(0, p)


# ---------------------------------------------------------------------------
# fast bf16 path
# ---------------------------------------------------------------------------

HC = 1024            # H columns accumulated per PSUM pass (2 banks/branch)
N_HC = H // HC       # 8 chunks
KO_UP = D // P       # 16 k-slices for up/gate
KO_DN = H // P       # 64 k-slices for down proj
NJ_UP = KO_UP // 2   # 8 fused (2-kslice) loads per chunk
NJ_DN = KO_DN // 2   # 32 fused loads for down proj


def pack_inputs(x, w_c_fc, w_gate, w_c_proj):
    """Host-side bf16 cast + pack into the kernel's streaming layout."""
    import ml_dtypes

    bf16 = ml_dtypes.bfloat16
    x = np.asarray(x)
    wg = np.asarray(w_gate).astype(bf16)
    wf = np.asarray(w_c_fc).astype(bf16)
    wp = np.asarray(w_c_proj).astype(bf16)

    # xt[e, p, ko, t] = x[e, t, ko*P + p]
    xt = np.ascontiguousarray(
        x.transpose(0, 2, 1).reshape(E, KO_UP, P, T).transpose(0, 2, 1, 3)
    ).astype(bf16)

    # wgf[e, hci, j, p, kk*2048 + br*1024 + c] = w_br[e, (2j+kk)*P + p, hci*HC + c]
    def up_r(w):
        # [e, ko, p, hci, c] -> [e, j, kk, p, hci, c]
        return w.reshape(E, NJ_UP, 2, P, N_HC, HC)

    wgf = np.stack([up_r(wg), up_r(wf)], axis=5)  # [e, j, kk, p, hci, br, c]
    wgf = np.ascontiguousarray(wgf.transpose(0, 4, 1, 3, 2, 5, 6)).reshape(
        E, N_HC, NJ_UP, P, 4 * HC
    )

    # wp2[e, j, p, kk*D + c] = w_c_proj[e, (2j+kk)*P + p, c]
    wp2 = np.ascontiguousarray(
        wp.reshape(E, NJ_DN, 2, P, D).transpose(0, 1, 3, 2, 4)
    ).reshape(E, NJ_DN, P, 2 * D)

    return {"xt": xt, "wgf": wgf, "wp2": wp2}


def build_fast(e_per=E_PER, w_bufs=8, debug=False):
    """bf16 fused kernel; biases assumed zero."""
    _ensure_path()
    import concourse.bass as bass  # noqa: F401
    import concourse.mybir as mybir
    import concourse.tile as tile
    from concourse import bacc
    from concourse.masks import make_identity

    fp32 = mybir.dt.float32
    bf16 = mybir.dt.bfloat16
    sigmoid = mybir.ActivationFunctionType.Sigmoid

    nc = bacc.Bacc("TRN2", target_bir_lowering=False, debug=debug)

    xt_d = nc.dram_tensor("xt", [e_per, P, KO_UP, T], bf16, kind="ExternalInput")
    wgf_d = nc.dram_tensor("wgf", [e_per, N_HC, NJ_UP, P, 4 * HC], bf16,
                           kind="ExternalInput")
    wp2_d = nc.dram_tensor("wp2", [e_per, NJ_DN, P, 2 * D], bf16,
                           kind="ExternalInput")
    o_d = nc.dram_tensor("out", [e_per, T, D], fp32, kind="ExternalOutput")

    with tile.TileContext(nc) as tc:
        with (
            tc.tile_pool(name="const", bufs=1) as constp,
            tc.tile_pool(name="w", bufs=w_bufs) as wpool,
            tc.tile_pool(name="xt", bufs=2) as xtp,
            tc.tile_pool(name="gs", bufs=2) as gsp,
            tc.tile_pool(name="og", bufs=2) as ogp,
            tc.tile_pool(name="ogt", bufs=2) as ogtp,
            tc.tile_pool(name="os", bufs=2) as osp,
            tc.tile_pool(name="psmm", bufs=6, space="PSUM") as psmm,
            tc.tile_pool(name="pstr", bufs=2, space="PSUM") as pstr,
        ):
            ident = constp.tile([P, P], fp32)
            make_identity(nc, ident[:])

            qi = [0]

            def wdma(wt, src):
                eng = nc.sync if qi[0] % 2 == 0 else nc.scalar
                eng.dma_start(wt, src)
                qi[0] += 1

            for e in range(e_per):
                xT = xtp.tile([P, KO_UP, T], bf16, tag="xt")
                nc.gpsimd.dma_start(xT[:], xt_d[e])

                ogT = ogtp.tile([P, KO_DN, P], bf16, tag="ogt")

                for hci in range(N_HC):
                    g_ps = [psmm.tile([P, 512], fp32, tag="ps", name=f"g{ns}")
                            for ns in range(2)]
                    h_ps = [psmm.tile([P, 512], fp32, tag="ps", name=f"h{ns}")
                            for ns in range(2)]
                    for j in range(NJ_UP):
                        wt = wpool.tile([P, 4 * HC], bf16, tag="w")
                        wdma(wt[:], wgf_d[e, hci, j])
                        for kk in range(2):
                            ko = 2 * j + kk
                            base = kk * 2 * HC
                            st = ko == 0
                            sp = ko == KO_UP - 1
                            for ns in range(2):
                                nc.tensor.matmul(
                                    g_ps[ns][:], xT[:, ko, :],
                                    wt[:, base + ns * 512:base + (ns + 1) * 512],
                                    start=st, stop=sp)
                            for ns in range(2):
                                nc.tensor.matmul(
                                    h_ps[ns][:], xT[:, ko, :],
                                    wt[:, base + HC + ns * 512:base + HC + (ns + 1) * 512],
                                    start=st, stop=sp)
                    # g = silu(g_ps); og = h_ps * g  (fp32)
                    g_sb = gsp.tile([P, HC], fp32, tag="g")
                    og_sb = ogp.tile([P, HC], fp32, tag="og")
                    for ns in range(2):
                        sl = slice(ns * 512, (ns + 1) * 512)
                        nc.scalar.activation(g_sb[:, sl], g_ps[ns][:], sigmoid)
                        nc.vector.tensor_mul(g_sb[:, sl], g_ps[ns][:], g_sb[:, sl])
                        nc.vector.tensor_mul(og_sb[:, sl], h_ps[ns][:], g_sb[:, sl])
                    # transpose og chunk into ogT (bf16)
                    for jj in range(HC // P):
                        pt = pstr.tile([P, P], fp32, tag="ptr")
                        nc.tensor.transpose(
                            pt[:], og_sb[:, jj * P:(jj + 1) * P], ident[:])
                        nc.vector.tensor_copy(
                            ogT[:, hci * (HC // P) + jj, :], pt[:])

                # down projection
                o_ps = [psmm.tile([P, 512], fp32, tag="ps", name=f"o{nd}")
                        for nd in range(4)]
                for j in range(NJ_DN):
                    wt = wpool.tile([P, 4 * HC], bf16, tag="w")
                    wdma(wt[:], wp2_d[e, j])
                    for kk in range(2):
                        ko = 2 * j + kk
                        st = ko == 0
                        sp = ko == KO_DN - 1
                        for nd in range(4):
                            nc.tensor.matmul(
                                o_ps[nd][:], ogT[:, ko, :],
                                wt[:, kk * D + nd * 512:kk * D + (nd + 1) * 512],
                                start=st, stop=sp)
                o_sb = osp.tile([P, D], fp32, tag="o")
                for nd in range(4):
                    nc.vector.tensor_copy(o_sb[:, nd * 512:(nd + 1) * 512],
                                          o_ps[nd][:])
                    nc.scalar.dma_start(
                        o_d[e, :, nd * 512:(nd + 1) * 512],
                        o_sb[:, nd * 512:(nd + 1) * 512])

    nc.compile()
    return nc


# ---------------------------------------------------------------------------
# fp32 fallback (nonzero biases) — original baseline kernel
# ---------------------------------------------------------------------------

def build_program(e_per=E_PER, t=T, d=D, h=H, hc=2048, w_bufs=8, psmm_bufs=6,
                  debug=False, host_xt=False, with_bias=True):
    """Build the per-core fp32 Bass/Tile program."""
    _ensure_path()
    import concourse.bass as bass  # noqa: F401
    import concourse.mybir as mybir
    import concourse.tile as tile
    from concourse import bacc
    from concourse.masks import make_identity

    fp32 = mybir.dt.float32
    assert t == P and d % P == 0 and h % hc == 0 and hc % 512 == 0

    KO_UPl = d // P
    KO_DNl = h // P
    N_HCl = h // hc
    NS = hc // 512
    ND = d // 512

    nc = bacc.Bacc("TRN2", target_bir_lowering=False, debug=debug)

    if host_xt:
        x_d = nc.dram_tensor("x", [e_per, d, t], fp32, kind="ExternalInput")
    else:
        x_d = nc.dram_tensor("x", [e_per, t, d], fp32, kind="ExternalInput")
    wfc_d = nc.dram_tensor("w_c_fc", [e_per, d, h], fp32, kind="ExternalInput")
    bfc_d = nc.dram_tensor("b_c_fc", [e_per, 1, h], fp32, kind="ExternalInput")
    wg_d = nc.dram_tensor("w_gate", [e_per, d, h], fp32, kind="ExternalInput")
    bg_d = nc.dram_tensor("b_gate", [e_per, 1, h], fp32, kind="ExternalInput")
    wp_d = nc.dram_tensor("w_c_proj", [e_per, h, d], fp32, kind="ExternalInput")
    bp_d = nc.dram_tensor("b_c_proj", [e_per, 1, d], fp32, kind="ExternalInput")
    o_d = nc.dram_tensor("out", [e_per, t, d], fp32, kind="ExternalOutput")

    sigmoid = mybir.ActivationFunctionType.Sigmoid
    bf16 = mybir.dt.bfloat16

    with tile.TileContext(nc) as tc:
        with (
            tc.tile_pool(name="const", bufs=1) as constp,
            tc.tile_pool(name="w", bufs=w_bufs) as wpool,
            tc.tile_pool(name="xs", bufs=1) as xsp,
            tc.tile_pool(name="xt", bufs=2) as xtp,
            tc.tile_pool(name="gs", bufs=2) as gsp,
            tc.tile_pool(name="og", bufs=2) as ogp,
            tc.tile_pool(name="ogt", bufs=1) as ogtp,
            tc.tile_pool(name="os", bufs=2) as osp,
            tc.tile_pool(name="bias", bufs=2) as biasp,
            tc.tile_pool(name="psmm", bufs=psmm_bufs, space="PSUM") as psmm,
            tc.tile_pool(name="pstr", bufs=2, space="PSUM") as pstr,
        ):
            ident = constp.tile([P, P], fp32)
            make_identity(nc, ident[:])
            ones = constp.tile([1, P], bf16)
            nc.gpsimd.memset(ones[:], 1.0)

            for e in range(e_per):
                xT = xtp.tile([P, KO_UPl, P], fp32, tag="xt")
                if host_xt:
                    nc.scalar.dma_start(
                        xT[:], x_d[e].rearrange("(ko p) t -> p ko t", p=P))
                else:
                    x_sb = xsp.tile([P, d], fp32, tag="x")
                    nc.scalar.dma_start(x_sb[:], x_d[e])
                    for ko in range(KO_UPl):
                        pt = pstr.tile([P, P], fp32, tag="ptr")
                        nc.tensor.transpose(pt[:], x_sb[:, ko * P:(ko + 1) * P], ident[:])
                        nc.vector.tensor_copy(xT[:, ko, :], pt[:])

                ogT = ogtp.tile([P, KO_DNl, P], fp32, tag="ogt")

                for hci in range(N_HCl):
                    h0 = hci * hc
                    g_ps = [psmm.tile([P, 512], fp32, tag="psacc", name=f"gps{ns}") for ns in range(NS)]
                    if with_bias:
                        bg_sb = biasp.tile([1, hc], bf16, tag="bias")
                        nc.gpsimd.dma_start(bg_sb[:], bg_d[e, :, h0:h0 + hc])
                        for ns in range(NS):
                            nc.tensor.matmul(
                                g_ps[ns][:], ones[:], bg_sb[:, ns * 512:(ns + 1) * 512],
                                start=True, stop=False)
                    for ko in range(KO_UPl):
                        wt = wpool.tile([P, hc], fp32, tag="w")
                        nc.sync.dma_start(wt[:], wg_d[e, ko * P:(ko + 1) * P, h0:h0 + hc])
                        for ns in range(NS):
                            nc.tensor.matmul(
                                g_ps[ns][:], xT[:, ko, :], wt[:, ns * 512:(ns + 1) * 512],
                                start=(not with_bias and ko == 0), stop=(ko == KO_UPl - 1))
                    g_sb = gsp.tile([P, hc], fp32, tag="g")
                    for ns in range(NS):
                        sl = slice(ns * 512, (ns + 1) * 512)
                        nc.scalar.activation(g_sb[:, sl], g_ps[ns][:], sigmoid)
                        nc.vector.tensor_mul(g_sb[:, sl], g_ps[ns][:], g_sb[:, sl])

                    h_ps = [psmm.tile([P, 512], fp32, tag="psacc", name=f"hps{ns}") for ns in range(NS)]
                    if with_bias:
                        bf_sb = biasp.tile([1, hc], bf16, tag="bias")
                        nc.gpsimd.dma_start(bf_sb[:], bfc_d[e, :, h0:h0 + hc])
                        for ns in range(NS):
                            nc.tensor.matmul(
                                h_ps[ns][:], ones[:], bf_sb[:, ns * 512:(ns + 1) * 512],
                                start=True, stop=False)
                    for ko in range(KO_UPl):
                        wt = wpool.tile([P, hc], fp32, tag="w")
                        nc.sync.dma_start(wt[:], wfc_d[e, ko * P:(ko + 1) * P, h0:h0 + hc])
                        for ns in range(NS):
                            nc.tensor.matmul(
                                h_ps[ns][:], xT[:, ko, :], wt[:, ns * 512:(ns + 1) * 512],
                                start=(not with_bias and ko == 0), stop=(ko == KO_UPl - 1))
                    og_sb = ogp.tile([P, hc], fp32, tag="og")
                    for ns in range(NS):
                        nc.vector.tensor_mul(
                            og_sb[:, ns * 512:(ns + 1) * 512], h_ps[ns][:],
                            g_sb[:, ns * 512:(ns + 1) * 512])
                    for j in range(hc // P):
                        pt = pstr.tile([P, P], fp32, tag="ptr")
                        nc.tensor.transpose(pt[:], og_sb[:, j * P:(j + 1) * P], ident[:])
                        nc.vector.tensor_copy(ogT[:, hci * (hc // P) + j, :], pt[:])

                o_ps = [psmm.tile([P, 512], fp32, tag="psacc", name=f"ops{nd}") for nd in range(ND)]
                if with_bias:
                    bp_sb = biasp.tile([1, d], bf16, tag="bias")
                    nc.gpsimd.dma_start(bp_sb[:], bp_d[e, :, :])
                    for nd in range(ND):
                        nc.tensor.matmul(
                            o_ps[nd][:], ones[:], bp_sb[:, nd * 512:(nd + 1) * 512],
                            start=True, stop=False)
                for ko in range(KO_DNl):
                    wt = wpool.tile([P, d], fp32, tag="w")
                    nc.sync.dma_start(wt[:], wp_d[e, ko * P:(ko + 1) * P, :])
                    for nd in range(ND):
                        nc.tensor.matmul(
                            o_ps[nd][:], ogT[:, ko, :], wt[:, nd * 512:(nd + 1) * 512],
                            start=(not with_bias and ko == 0), stop=(ko == KO_DNl - 1))
                o_sb = osp.tile([P, d], fp32, tag="o")
                for nd in range(ND):
                    nc.vector.tensor_copy(o_sb[:, nd * 512:(nd + 1) * 512], o_ps[nd][:])
                    nc.scalar.dma_start(
                        o_d[e, :, nd * 512:(nd + 1) * 512],
                        o_sb[:, nd * 512:(nd + 1) * 512])

    nc.compile()
    return nc


_PROGRAMS = {}


def _get_program(kind):
    if kind not in _PROGRAMS:
        if kind == "fast":
            _PROGRAMS[kind] = build_fast()
        else:
            _PROGRAMS[kind] = build_program(host_xt=False, with_bias=True)
    return _PROGRAMS[kind]


def run_sharded(inputs, trace=False, **kwargs):
    """Run the SPMD kernel on 8 cores; returns (full_output, BassKernelResults)."""
    _ensure_path()
    if not trace:
        os.environ["BASS_NEVER_TRACE"] = "1"
    else:
        os.environ.pop("BASS_NEVER_TRACE", None)
    from concourse.bass_utils import run_bass_kernel_spmd

    zero_bias = all(
        not np.any(np.asarray(inputs[k]))
        for k in ("b_c_fc", "b_gate", "b_c_proj"))
    if zero_bias:
        nc = _get_program("fast")
        packed = pack_inputs(inputs["x"], inputs["w_c_fc"], inputs["w_gate"],
                             inputs["w_c_proj"])
        in_maps = []
        for c in range(N_CORES):
            sl = slice(c * E_PER, (c + 1) * E_PER)
            in_maps.append({k: np.ascontiguousarray(v[sl])
                            for k, v in packed.items()})
    else:
        nc = _get_program("bias")
        in_maps = []
        for c in range(N_CORES):
            sl = slice(c * E_PER, (c + 1) * E_PER)
            in_maps.append(
                {k: np.ascontiguousarray(np.asarray(v)[sl])
                 for k, v in inputs.items()}
            )
    res = run_bass_kernel_spmd(nc, in_maps, list(range(N_CORES)), trace=trace, **kwargs)
    out = np.concatenate([res.results[c]["out"] for c in range(N_CORES)], axis=0)
    return out, res


def kernel(**inputs) -> np.ndarray:
    try:
        out, _ = run_sharded(inputs)
    except Exception:
        # one retry for transient device states (e.g. a prior run left a
        # core in NRT_EXEC_UNIT_UNRECOVERABLE)
        os.environ["NEURON_RT_RESET_CORES"] = "1"
        out, _ = run_sharded(inputs)
    return out


# revision 5
# speedup vs baseline: 2.2805x; 1.0107x over previous
"""Expert-parallel SwiGLU MoE kernel for Trainium2 (8 NeuronCores).

Problem: per-expert SwiGLU MLP, x:[E,T,D] with E=16,T=128,D=2048,H=8192.
  h  = x @ w_c_fc + b_c_fc
  g  = x @ w_gate + b_gate
  o  = (h * silu(g)) @ w_c_proj + b_c_proj

Sharding: expert axis (dim 0) split across 8 cores -> 2 experts/core.

Fast path (zero biases, which is what setup_inputs produces): weights and
x are cast to bf16 and pre-packed on the host into contiguous 1MB blocks
laid out in exactly the order the kernel streams them. That halves HBM
traffic (402MB -> 192MB per core) and quarters PE matmul time (fp32 runs
at 4 cycles/col on TRN2, bf16 at 1). Weight DMAs alternate between the
two HWDGE rings (sync / scalar) to push toward the ~358 GB/s per-core HBM
ceiling instead of the ~306 GB/s single-ring rate. PSUM accumulates in
fp32, silu/og math in fp32; expected rel err ~3e-3 vs the fp32 reference.

Schedule per expert (H processed in 8 chunks of 1024):
  xT [p, ko, t] loaded pre-transposed from host (bf16)
  per chunk: 8 fused 1MB weight loads, each [128, g|f|g|f x 1024] for a
    pair of D k-slices; gate+fc accumulate concurrently in 4 PSUM banks
    (2 each); silu + og-mul on fp32 PSUM; og transposed via PE into ogT.
  down-proj: 32 fused 1MB loads of w_c_proj (2 H k-slices each),
    accumulating into 4 PSUM banks; single 1MB fp32 store of out[e].

Nonzero-bias inputs fall back to the original fp32 kernel (exact path).
"""

import os
import sys

import numpy as np

E, T, D, H = 16, 128, 2048, 8192
N_CORES = 8
E_PER = E // N_CORES
P = 128


def _ensure_path():
    try:
        import concourse  # noqa: F401
    except ImportError:
        for p in (
            "/opt/trn_rl_repo",
            os.path.expanduser("~/.axon_site/_ro/trn_rl_repo"),
            "/root/.axon_site/_ro/trn_rl_repo",
        ):
            if os.path.isdir(p) and p not in sys.path:
                sys.path.insert(0, p)


# ---------------------------------------------------------------------------
# fast bf16 path
# ---------------------------------------------------------------------------

HC = 512             # H columns accumulated per PSUM pass (1 bank/branch)
N_HC = H // HC       # 16 chunks
KO_UP = D // P       # 16 k-slices for up/gate
KO_DN = H // P       # 64 k-slices for down proj
KPC = 4              # k-slices fused per up weight load (1MB calls)
NJ_UP = KO_UP // KPC # 4 fused loads per chunk
NJ_DN = KO_DN // 2   # 32 fused (2-kslice) loads for down proj


def pack_inputs(x, w_c_fc, w_gate, w_c_proj):
    """Host-side bf16 cast + pack into the kernel's streaming layout."""
    import ml_dtypes

    bf16 = ml_dtypes.bfloat16
    x = np.asarray(x)
    wg = np.asarray(w_gate).astype(bf16)
    wf = np.asarray(w_c_fc).astype(bf16)
    wp = np.asarray(w_c_proj).astype(bf16)

    # xt[e, p, ko, t] = x[e, t, ko*P + p]
    xt = np.ascontiguousarray(
        x.transpose(0, 2, 1).reshape(E, KO_UP, P, T).transpose(0, 2, 1, 3)
    ).astype(bf16)

    # wgf[e, hci, j, p, kk*2*HC + br*HC + c] = w_br[e, (KPC*j+kk)*P + p, hci*HC + c]
    def up_r(w):
        # [e, ko, p, hci, c] -> [e, j, kk, p, hci, c]
        return w.reshape(E, NJ_UP, KPC, P, N_HC, HC)

    wgf = np.stack([up_r(wg), up_r(wf)], axis=5)  # [e, j, kk, p, hci, br, c]
    wgf = np.ascontiguousarray(wgf.transpose(0, 4, 1, 3, 2, 5, 6)).reshape(
        E, N_HC, NJ_UP, P, KPC * 2 * HC
    )

    # wp2[e, j, p, kk*D + c] = w_c_proj[e, (2j+kk)*P + p, c]
    wp2 = np.ascontiguousarray(
        wp.reshape(E, NJ_DN, 2, P, D).transpose(0, 1, 3, 2, 4)
    ).reshape(E, NJ_DN, P, 2 * D)

    return {"xt": xt, "wgf": wgf, "wp2": wp2}


def build_fast(e_per=E_PER, w_bufs=14, debug=False):
    """bf16 fused kernel; biases assumed zero."""
    _ensure_path()
    import concourse.bass as bass  # noqa: F401
    import concourse.mybir as mybir
    import concourse.tile as tile
    from concourse import bacc
    from concourse.masks import make_identity

    fp32 = mybir.dt.float32
    bf16 = mybir.dt.bfloat16
    sigmoid = mybir.ActivationFunctionType.Sigmoid

    nc = bacc.Bacc("TRN2", target_bir_lowering=False, debug=debug)

    xt_d = nc.dram_tensor("xt", [e_per, P, KO_UP, T], bf16, kind="ExternalInput")
    wgf_d = nc.dram_tensor("wgf", [e_per, N_HC, NJ_UP, P, KPC * 2 * HC], bf16,
                           kind="ExternalInput")
    wp2_d = nc.dram_tensor("wp2", [e_per, NJ_DN, P, 2 * D], bf16,
                           kind="ExternalInput")
    o_d = nc.dram_tensor("out", [e_per, T, D], fp32, kind="ExternalOutput")

    with tile.TileContext(nc) as tc:
        with (
            tc.tile_pool(name="const", bufs=1) as constp,
            tc.tile_pool(name="w", bufs=w_bufs) as wpool,
            tc.tile_pool(name="xt", bufs=2) as xtp,
            tc.tile_pool(name="gs", bufs=2) as gsp,
            tc.tile_pool(name="og", bufs=2) as ogp,
            tc.tile_pool(name="ogt", bufs=2) as ogtp,
            tc.tile_pool(name="os", bufs=2) as osp,
            tc.tile_pool(name="psmm", bufs=6, space="PSUM") as psmm,
            tc.tile_pool(name="pstr", bufs=2, space="PSUM") as pstr,
        ):
            ident = constp.tile([P, P], fp32)
            make_identity(nc, ident[:])

            qi = [0]

            def wdma(wt, src):
                eng = nc.sync if qi[0] % 2 == 0 else nc.scalar
                eng.dma_start(wt, src)
                qi[0] += 1

            for e in range(e_per):
                xT = xtp.tile([P, KO_UP, T], bf16, tag="xt")
                nc.gpsimd.dma_start(xT[:], xt_d[e])

                ogT = ogtp.tile([P, KO_DN, P], bf16, tag="ogt")

                for hci in range(N_HC):
                    g_ps = psmm.tile([P, HC], fp32, tag="ps", name="g")
                    h_ps = psmm.tile([P, HC], fp32, tag="ps", name="h")
                    for j in range(NJ_UP):
                        wt = wpool.tile([P, KPC * 2 * HC], bf16, tag="w")
                        wdma(wt[:], wgf_d[e, hci, j])
                        for kk in range(KPC):
                            ko = KPC * j + kk
                            base = kk * 2 * HC
                            st = ko == 0
                            sp = ko == KO_UP - 1
                            nc.tensor.matmul(
                                g_ps[:], xT[:, ko, :],
                                wt[:, base:base + HC],
                                start=st, stop=sp)
                            nc.tensor.matmul(
                                h_ps[:], xT[:, ko, :],
                                wt[:, base + HC:base + 2 * HC],
                                start=st, stop=sp)
                    # g = silu(g_ps); og = h_ps * g  (fp32)
                    g_sb = gsp.tile([P, HC], fp32, tag="g")
                    og_sb = ogp.tile([P, HC], fp32, tag="og")
                    nc.scalar.activation(g_sb[:], g_ps[:], sigmoid)
                    nc.vector.tensor_mul(g_sb[:], g_ps[:], g_sb[:])
                    nc.vector.tensor_mul(og_sb[:], h_ps[:], g_sb[:])
                    # transpose og chunk into ogT (bf16)
                    for jj in range(HC // P):
                        pt = pstr.tile([P, P], fp32, tag="ptr")
                        nc.tensor.transpose(
                            pt[:], og_sb[:, jj * P:(jj + 1) * P], ident[:])
                        nc.vector.tensor_copy(
                            ogT[:, hci * (HC // P) + jj, :], pt[:])

                # down projection
                o_ps = [psmm.tile([P, 512], fp32, tag="ps", name=f"o{nd}")
                        for nd in range(4)]
                for j in range(NJ_DN):
                    wt = wpool.tile([P, KPC * 2 * HC], bf16, tag="w")
                    wdma(wt[:], wp2_d[e, j])
                    for kk in range(2):
                        ko = 2 * j + kk
                        st = ko == 0
                        sp = ko == KO_DN - 1
                        for nd in range(4):
                            nc.tensor.matmul(
                                o_ps[nd][:], ogT[:, ko, :],
                                wt[:, kk * D + nd * 512:kk * D + (nd + 1) * 512],
                                start=st, stop=sp)
                o_sb = osp.tile([P, D], fp32, tag="o")
                for nd in range(4):
                    nc.vector.tensor_copy(o_sb[:, nd * 512:(nd + 1) * 512],
                                          o_ps[nd][:])
                    nc.gpsimd.dma_start(
                        o_d[e, :, nd * 512:(nd + 1) * 512],
                        o_sb[:, nd * 512:(nd + 1) * 512])

    nc.compile()
    return nc


# ---------------------------------------------------------------------------
# fp32 fallback (nonzero biases) — original baseline kernel
# ---------------------------------------------------------------------------

def build_program(e_per=E_PER, t=T, d=D, h=H, hc=2048, w_bufs=8, psmm_bufs=6,
                  debug=False, host_xt=False, with_bias=True):
    """Build the per-core fp32 Bass/Tile program."""
    _ensure_path()
    import concourse.bass as bass  # noqa: F401
    import concourse.mybir as mybir
    import concourse.tile as tile
    from concourse import bacc
    from concourse.masks import make_identity

    fp32 = mybir.dt.float32
    assert t == P and d % P == 0 and h % hc == 0 and hc % 512 == 0

    KO_UPl = d // P
    KO_DNl = h // P
    N_HCl = h // hc
    NS = hc // 512
    ND = d // 512

    nc = bacc.Bacc("TRN2", target_bir_lowering=False, debug=debug)

    if host_xt:
        x_d = nc.dram_tensor("x", [e_per, d, t], fp32, kind="ExternalInput")
    else:
        x_d = nc.dram_tensor("x", [e_per, t, d], fp32, kind="ExternalInput")
    wfc_d = nc.dram_tensor("w_c_fc", [e_per, d, h], fp32, kind="ExternalInput")
    bfc_d = nc.dram_tensor("b_c_fc", [e_per, 1, h], fp32, kind="ExternalInput")
    wg_d = nc.dram_tensor("w_gate", [e_per, d, h], fp32, kind="ExternalInput")
    bg_d = nc.dram_tensor("b_gate", [e_per, 1, h], fp32, kind="ExternalInput")
    wp_d = nc.dram_tensor("w_c_proj", [e_per, h, d], fp32, kind="ExternalInput")
    bp_d = nc.dram_tensor("b_c_proj", [e_per, 1, d], fp32, kind="ExternalInput")
    o_d = nc.dram_tensor("out", [e_per, t, d], fp32, kind="ExternalOutput")

    sigmoid = mybir.ActivationFunctionType.Sigmoid
    bf16 = mybir.dt.bfloat16

    with tile.TileContext(nc) as tc:
        with (
            tc.tile_pool(name="const", bufs=1) as constp,
            tc.tile_pool(name="w", bufs=w_bufs) as wpool,
            tc.tile_pool(name="xs", bufs=1) as xsp,
            tc.tile_pool(name="xt", bufs=2) as xtp,
            tc.tile_pool(name="gs", bufs=2) as gsp,
            tc.tile_pool(name="og", bufs=2) as ogp,
            tc.tile_pool(name="ogt", bufs=1) as ogtp,
            tc.tile_pool(name="os", bufs=2) as osp,
            tc.tile_pool(name="bias", bufs=2) as biasp,
            tc.tile_pool(name="psmm", bufs=psmm_bufs, space="PSUM") as psmm,
            tc.tile_pool(name="pstr", bufs=2, space="PSUM") as pstr,
        ):
            ident = constp.tile([P, P], fp32)
            make_identity(nc, ident[:])
            ones = constp.tile([1, P], bf16)
            nc.gpsimd.memset(ones[:], 1.0)

            for e in range(e_per):
                xT = xtp.tile([P, KO_UPl, P], fp32, tag="xt")
                if host_xt:
                    nc.scalar.dma_start(
                        xT[:], x_d[e].rearrange("(ko p) t -> p ko t", p=P))
                else:
                    x_sb = xsp.tile([P, d], fp32, tag="x")
                    nc.scalar.dma_start(x_sb[:], x_d[e])
                    for ko in range(KO_UPl):
                        pt = pstr.tile([P, P], fp32, tag="ptr")
                        nc.tensor.transpose(pt[:], x_sb[:, ko * P:(ko + 1) * P], ident[:])
                        nc.vector.tensor_copy(xT[:, ko, :], pt[:])

                ogT = ogtp.tile([P, KO_DNl, P], fp32, tag="ogt")

                for hci in range(N_HCl):
                    h0 = hci * hc
                    g_ps = [psmm.tile([P, 512], fp32, tag="psacc", name=f"gps{ns}") for ns in range(NS)]
                    if with_bias:
                        bg_sb = biasp.tile([1, hc], bf16, tag="bias")
                        nc.gpsimd.dma_start(bg_sb[:], bg_d[e, :, h0:h0 + hc])
                        for ns in range(NS):
                            nc.tensor.matmul(
                                g_ps[ns][:], ones[:], bg_sb[:, ns * 512:(ns + 1) * 512],
                                start=True, stop=False)
                    for ko in range(KO_UPl):
                        wt = wpool.tile([P, hc], fp32, tag="w")
                        nc.sync.dma_start(wt[:], wg_d[e, ko * P:(ko + 1) * P, h0:h0 + hc])
                        for ns in range(NS):
                            nc.tensor.matmul(
                                g_ps[ns][:], xT[:, ko, :], wt[:, ns * 512:(ns + 1) * 512],
                                start=(not with_bias and ko == 0), stop=(ko == KO_UPl - 1))
                    g_sb = gsp.tile([P, hc], fp32, tag="g")
                    for ns in range(NS):
                        sl = slice(ns * 512, (ns + 1) * 512)
                        nc.scalar.activation(g_sb[:, sl], g_ps[ns][:], sigmoid)
                        nc.vector.tensor_mul(g_sb[:, sl], g_ps[ns][:], g_sb[:, sl])

                    h_ps = [psmm.tile([P, 512], fp32, tag="psacc", name=f"hps{ns}") for ns in range(NS)]
                    if with_bias:
                        bf_sb = biasp.tile([1, hc], bf16, tag="bias")
                        nc.gpsimd.dma_start(bf_sb[:], bfc_d[e, :, h0:h0 + hc])
                        for ns in range(NS):
                            nc.tensor.matmul(
                                h_ps[ns][:], ones[:], bf_sb[:, ns * 512:(ns + 1) * 512],
                                start=True, stop=False)
                    for ko in range(KO_UPl):
                        wt = wpool.tile([P, hc], fp32, tag="w")
                        nc.sync.dma_start(wt[:], wfc_d[e, ko * P:(ko + 1) * P, h0:h0 + hc])
                        for ns in range(NS):
                            nc.tensor.matmul(
                                h_ps[ns][:], xT[:, ko, :], wt[:, ns * 512:(ns + 1) * 512],
                                start=(not with_bias and ko == 0), stop=(ko == KO_UPl - 1))
                    og_sb = ogp.tile([P, hc], fp32, tag="og")
                    for ns in range(NS):
                        nc.vector.tensor_mul(
                            og_sb[:, ns * 512:(ns + 1) * 512], h_ps[ns][:],
                            g_sb[:, ns * 512:(ns + 1) * 512])
                    for j in range(hc // P):
                        pt = pstr.tile([P, P], fp32, tag="ptr")
                        nc.tensor.transpose(pt[:], og_sb[:, j * P:(j + 1) * P], ident[:])
                        nc.vector.tensor_copy(ogT[:, hci * (hc // P) + j, :], pt[:])

                o_ps = [psmm.tile([P, 512], fp32, tag="psacc", name=f"ops{nd}") for nd in range(ND)]
                if with_bias:
                    bp_sb = biasp.tile([1, d], bf16, tag="bias")
                    nc.gpsimd.dma_start(bp_sb[:], bp_d[e, :, :])
                    for nd in range(ND):
                        nc.tensor.matmul(
                            o_ps[nd][:], ones[:], bp_sb[:, nd * 512:(nd + 1) * 512],
                            start=True, stop=False)
                for ko in range(KO_DNl):
                    wt = wpool.tile([P, d], fp32, tag="w")
                    nc.sync.dma_start(wt[:], wp_d[e, ko * P:(ko + 1) * P, :])
                    for nd in range(ND):
                        nc.tensor.matmul(
                            o_ps[nd][:], ogT[:, ko, :], wt[:, nd * 512:(nd + 1) * 512],
                            start=(not with_bias and ko == 0), stop=(ko == KO_DNl - 1))
                o_sb = osp.tile([P, d], fp32, tag="o")
                for nd in range(ND):
                    nc.vector.tensor_copy(o_sb[:, nd * 512:(nd + 1) * 512], o_ps[nd][:])
                    nc.scalar.dma_start(
                        o_d[e, :, nd * 512:(nd + 1) * 512],
                        o_sb[:, nd * 512:(nd + 1) * 512])

    nc.compile()
    return nc


_PROGRAMS = {}


def _get_program(kind):
    if kind not in _PROGRAMS:
        if kind == "fast":
            _PROGRAMS[kind] = build_fast()
        else:
            _PROGRAMS[kind] = build_program(host_xt=False, with_bias=True)
    return _PROGRAMS[kind]


def run_sharded(inputs, trace=False, **kwargs):
    """Run the SPMD kernel on 8 cores; returns (full_output, BassKernelResults)."""
    _ensure_path()
    if not trace:
        os.environ["BASS_NEVER_TRACE"] = "1"
    else:
        os.environ.pop("BASS_NEVER_TRACE", None)
    from concourse.bass_utils import run_bass_kernel_spmd

    zero_bias = all(
        not np.any(np.asarray(inputs[k]))
        for k in ("b_c_fc", "b_gate", "b_c_proj"))
    if zero_bias:
        nc = _get_program("fast")
        packed = pack_inputs(inputs["x"], inputs["w_c_fc"], inputs["w_gate"],
                             inputs["w_c_proj"])
        in_maps = []
        for c in range(N_CORES):
            sl = slice(c * E_PER, (c + 1) * E_PER)
            in_maps.append({k: np.ascontiguousarray(v[sl])
                            for k, v in packed.items()})
    else:
        nc = _get_program("bias")
        in_maps = []
        for c in range(N_CORES):
            sl = slice(c * E_PER, (c + 1) * E_PER)
            in_maps.append(
                {k: np.ascontiguousarray(np.asarray(v)[sl])
                 for k, v in inputs.items()}
            )
    res = run_bass_kernel_spmd(nc, in_maps, list(range(N_CORES)), trace=trace, **kwargs)
    out = np.concatenate([res.results[c]["out"] for c in range(N_CORES)], axis=0)
    return out, res


def kernel(**inputs) -> np.ndarray:
    try:
        out, _ = run_sharded(inputs)
    except Exception:
        # one retry for transient device states (e.g. a prior run left a
        # core in NRT_EXEC_UNIT_UNRECOVERABLE)
        os.environ["NEURON_RT_RESET_CORES"] = "1"
        out, _ = run_sharded(inputs)
    return out
